# revision 1
# baseline (speedup 1.0000x reference)
"""Trainium2 Bass kernel for nn_Classifier_8461085573484 (2-layer GCN classifier).

Math: with x [N,1] and b1=0 (structurally true for this problem), both GCN
layers collapse to scalar per-node quantities:
  deg_d = indeg(d)+1;  dinv = 1/sqrt(deg);  u = x*dinv
  s_d   = sum_{e->d} u[src];   t = dinv*(s + x*dinv);  y = t*dinv
  sp_d  = sum_{e->d} relu(y[src]);  sm_d = sum_{e->d} relu(-y[src])
  alpha = dinv*(sp + relu(y));      beta = dinv*(sm + relu(-y))
  out2  = relu(alpha a^T + beta b^T + b2), a = relu(W1)@W2, b = relu(-W1)@W2
  logits = mean(out2) @ Wl + bl -> log_softmax.

Sharding (8 NeuronCores): NC k owns node range [12544k, 12544(k+1)).
Edges are routed host-side to (NC, lane) twice: by dst (scatter layout) and by
src (gather layout); lane = local_node % 128, q-code = local_node // 128 (98
bins/lane).  On device, segment sums are one-hot (98-wide is_equal vs iota)
matmuls with an identity lhsT accumulating in PSUM; gathers are one-hot
mult+reduce against the lane's 98-entry table slice.  The host only routes /
permutes per-edge values between the two layouts (no arithmetic) and applies
the O(1) classifier head.
"""
import contextlib
import ctypes
import sys
import types

import numpy as np

from concourse import bacc, bass, mybir
import concourse.tile as tile
from concourse import bass_utils

P = 128
Q = 98
NSH = P * Q            # 12544 nodes per NC shard
NC = 8
NPAD = NSH * NC        # 100352
N = 100000
F32 = mybir.dt.float32
BF16 = mybir.dt.bfloat16
PADQ = 127.0           # q-code for padding slots (never matches iota 0..97)
COLB = 256             # column padding granularity


def _install_ntff_shim():
    """Provide antenv.axon_hooks so run_bass_kernel_spmd(trace=True) works."""
    if "antenv.axon_hooks" in sys.modules:
        return
    import antenv

    _hook = None
    try:
        lib = ctypes.CDLL("/opt/axon/libaxon_pjrt.so")
        if hasattr(lib, "axon_start_nrt_profile"):
            lib.axon_start_nrt_profile.argtypes = [
                ctypes.POINTER(ctypes.c_int64), ctypes.c_size_t]
            lib.axon_start_nrt_profile.restype = ctypes.c_int64
            lib.axon_stop_nrt_profile.argtypes = [ctypes.c_char_p]
            lib.axon_stop_nrt_profile.restype = ctypes.c_int64

            @contextlib.contextmanager
            def _hook_impl(output_dir, device_ids):
                import jax
                jax.devices()
                if device_ids:
                    ids = (ctypes.c_int64 * len(device_ids))(*device_ids)
                    rc = lib.axon_start_nrt_profile(ids, len(device_ids))
                else:
                    rc = lib.axon_start_nrt_profile(None, 0)
                if rc != 0:
                    raise RuntimeError(f"axon_start_nrt_profile rc={rc}")
                try:
                    yield
                finally:
                    n = lib.axon_stop_nrt_profile(str(output_dir).encode())
                    if n < 0:
                        raise RuntimeError(f"axon_stop_nrt_profile rc={n}")

            _hook = _hook_impl
    except OSError:
        pass

    mod = types.ModuleType("antenv.axon_hooks")
    mod._hook = _hook
    mod.get_axon_ntff_profile_hook = lambda: mod._hook

    def set_axon_ntff_profile_hook(h):
        mod._hook = h

    mod.set_axon_ntff_profile_hook = set_axon_ntff_profile_hook
    sys.modules["antenv.axon_hooks"] = mod
    antenv.axon_hooks = mod


_install_ntff_shim()


# ---------------- host routing (sharding/layout only, no arithmetic) -------

def _build_layout(key_nodes):
    k = key_nodes // NSH
    loc = key_nodes - k * NSH
    lane = loc % P
    q = loc >> 7
    bucket = k.astype(np.int64) * P + lane
    order = np.argsort(bucket, kind="stable")
    counts = np.bincount(bucket, minlength=NC * P)
    C = int(np.ceil(max(counts.max(), 1) / COLB) * COLB)
    starts = np.zeros(NC * P, np.int64)
    starts[1:] = np.cumsum(counts)[:-1]
    slot = np.empty(key_nodes.shape[0], np.int64)
    slot[order] = np.arange(key_nodes.shape[0]) - starts[bucket[order]]
    flat = bucket * C + slot
    return C, flat, q


def _stage_qcodes(C, flat, q):
    import ml_dtypes
    arr = np.full(NC * P * C, PADQ, np.float32)
    arr[flat] = q.astype(np.float32)
    return np.ascontiguousarray(
        arr.reshape(NC, P, C).astype(ml_dtypes.bfloat16))


def _grid_of(vec_padded):
    return np.ascontiguousarray(vec_padded.reshape(NC, Q, P).transpose(0, 2, 1))


def _const_inputs():
    import ml_dtypes
    iota = np.tile(np.arange(Q, dtype=np.float32), (P, 1)).astype(ml_dtypes.bfloat16)
    ident = np.eye(P, dtype=np.float32).astype(ml_dtypes.bfloat16)
    return {"iota": iota, "ident": ident}


# ---------------- device phase builders ----------------

def _consts(nc):
    iota = nc.dram_tensor("iota", [P, Q], BF16, kind="ExternalInput")
    ident = nc.dram_tensor("ident", [P, P], BF16, kind="ExternalInput")
    return iota, ident


def _eq_col(nc, eq, iota_sb, q_sb, j):
    nc.vector.tensor_tensor(
        out=eq[:], in0=iota_sb[:],
        in1=q_sb[:, j:j + 1].to_broadcast([P, Q]),
        op=mybir.AluOpType.is_equal)


def build_k1(CD):
    """deg one-hot scatter -> dinv, u grids."""
    nc = bacc.Bacc("TRN2", target_bir_lowering=False, debug=False)
    dq = nc.dram_tensor("dq", [P, CD], BF16, kind="ExternalInput")
    iota, ident = _consts(nc)
    xg = nc.dram_tensor("xg", [P, Q], F32, kind="ExternalInput")
    dinv_o = nc.dram_tensor("dinv", [P, Q], F32, kind="ExternalOutput")
    u_o = nc.dram_tensor("u", [P, Q], F32, kind="ExternalOutput")
    with tile.TileContext(nc) as tc:
        with tc.tile_pool(name="sb", bufs=1) as pool, \
             tc.tile_pool(name="eqp", bufs=4) as eqpool, \
             tc.tile_pool(name="ps", bufs=1, space="PSUM") as psp:
            dq_sb = pool.tile([P, CD], BF16, tag="dq")
            iota_sb = pool.tile([P, Q], BF16, tag="iota")
            ident_sb = pool.tile([P, P], BF16, tag="ident")
            xg_sb = pool.tile([P, Q], F32, tag="xg")
            nc.sync.dma_start(dq_sb[:], dq.ap())
            nc.sync.dma_start(iota_sb[:], iota.ap())
            nc.sync.dma_start(ident_sb[:], ident.ap())
            nc.sync.dma_start(xg_sb[:], xg.ap())
            psC = psp.tile([P, Q], F32, space="PSUM")
            B = 8
            iota3 = iota_sb[:].rearrange("p (one q) -> p one q",
                                         one=1).to_broadcast([P, B, Q])
            for j0 in range(0, CD, B):
                eq = eqpool.tile([P, B * Q], BF16, tag="eq")
                eqv3 = eq[:].rearrange("p (b q) -> p b q", b=B)
                qcb = dq_sb[:, j0:j0 + B].rearrange(
                    "p (b one) -> p b one", one=1).to_broadcast([P, B, Q])
                nc.vector.tensor_tensor(out=eqv3, in0=iota3, in1=qcb,
                                        op=mybir.AluOpType.is_equal)
                for b in range(B):
                    j = j0 + b
                    nc.tensor.matmul(out=psC[:], lhsT=ident_sb[:],
                                     rhs=eq[:, b * Q:(b + 1) * Q],
                                     start=(j == 0), stop=(j == CD - 1))
            dinv_sb = pool.tile([P, Q], F32, tag="dinv")
            u_sb = pool.tile([P, Q], F32, tag="u")
            nc.scalar.activation(out=u_sb[:], in_=psC[:],
                                 func=mybir.ActivationFunctionType.Sqrt,
                                 bias=1.0, scale=1.0)
            nc.vector.reciprocal(out=dinv_sb[:], in_=u_sb[:])
            nc.vector.tensor_tensor(out=u_sb[:], in0=xg_sb[:], in1=dinv_sb[:],
                                    op=mybir.AluOpType.mult)
            nc.sync.dma_start(dinv_o.ap(), dinv_sb[:])
            nc.sync.dma_start(u_o.ap(), u_sb[:])
    nc.compile()
    return nc


def build_k2(CS):
    """one-hot gather: m0[p, j] = tab0[p, sq[p, j]] (0 for pad cols)."""
    nc = bacc.Bacc("TRN2", target_bir_lowering=False, debug=False)
    sq = nc.dram_tensor("sq", [P, CS], BF16, kind="ExternalInput")
    iota, _ = _consts(nc)
    tab0 = nc.dram_tensor("tab0", [P, Q], F32, kind="ExternalInput")
    m0 = nc.dram_tensor("m0", [P, CS], F32, kind="ExternalOutput")
    with tile.TileContext(nc) as tc:
        with tc.tile_pool(name="sb", bufs=1) as pool, \
             tc.tile_pool(name="eqp", bufs=4) as eqpool:
            sq_sb = pool.tile([P, CS], BF16, tag="sq")
            iota_sb = pool.tile([P, Q], BF16, tag="iota")
            tab_sb = pool.tile([P, Q], F32, tag="tab0")
            om = pool.tile([P, CS], F32, tag="om")
            nc.sync.dma_start(sq_sb[:], sq.ap())
            nc.sync.dma_start(iota_sb[:], iota.ap())
            nc.sync.dma_start(tab_sb[:], tab0.ap())
            B = 8
            iota3 = iota_sb[:].rearrange("p (one q) -> p one q",
                                         one=1).to_broadcast([P, B, Q])
            tab3 = tab_sb[:].rearrange("p (one q) -> p one q",
                                       one=1).to_broadcast([P, B, Q])
            for j0 in range(0, CS, B):
                eq = eqpool.tile([P, B * Q], F32, tag="eq")
                eqv3 = eq[:].rearrange("p (b q) -> p b q", b=B)
                qcb = sq_sb[:, j0:j0 + B].rearrange(
                    "p (b one) -> p b one", one=1).to_broadcast([P, B, Q])
                nc.vector.tensor_tensor(out=eqv3, in0=iota3, in1=qcb,
                                        op=mybir.AluOpType.is_equal)
                scr = eqpool.tile([P, B * Q], F32, tag="scr")
                scr3 = scr[:].rearrange("p (b q) -> p b q", b=B)
                nc.vector.tensor_tensor(out=scr3, in0=eqv3, in1=tab3,
                                        op=mybir.AluOpType.mult)
                nc.vector.tensor_reduce(out=om[:, j0:j0 + B], in_=scr3,
                                        axis=mybir.AxisListType.X,
                                        op=mybir.AluOpType.add)
            nc.sync.dma_start(m0.ap(), om[:])
    nc.compile()
    return nc


def build_k3(CD):
    """scatter s = segsum(msg by dst); node math -> yp, ym, y grids."""
    nc = bacc.Bacc("TRN2", target_bir_lowering=False, debug=False)
    dq = nc.dram_tensor("dq", [P, CD], BF16, kind="ExternalInput")
    vD = nc.dram_tensor("vD", [P, CD], F32, kind="ExternalInput")
    iota, ident = _consts(nc)
    dinv = nc.dram_tensor("dinvg", [P, Q], F32, kind="ExternalInput")
    xg = nc.dram_tensor("xg", [P, Q], F32, kind="ExternalInput")
    y_o = nc.dram_tensor("yg", [P, Q], F32, kind="ExternalOutput")
    with tile.TileContext(nc) as tc:
        with tc.tile_pool(name="sb", bufs=1) as pool, \
             tc.tile_pool(name="eqp", bufs=6) as eqpool, \
             tc.tile_pool(name="ps", bufs=1, space="PSUM") as psp:
            dq_sb = pool.tile([P, CD], BF16, tag="dq")
            vD_sb = pool.tile([P, CD], F32, tag="vD")
            iota_sb = pool.tile([P, Q], BF16, tag="iota")
            ident_sb = pool.tile([P, P], BF16, tag="ident")
            dinv_sb = pool.tile([P, Q], F32, tag="dinv")
            xg_sb = pool.tile([P, Q], F32, tag="xg")
            for t_sb, t in ((dq_sb, dq), (vD_sb, vD), (iota_sb, iota),
                            (ident_sb, ident), (dinv_sb, dinv), (xg_sb, xg)):
                nc.sync.dma_start(t_sb[:], t.ap())
            psS = psp.tile([P, Q], F32, space="PSUM")
            B = 8
            iota3 = iota_sb[:].rearrange("p (one q) -> p one q",
                                         one=1).to_broadcast([P, B, Q])
            for j0 in range(0, CD, B):
                eq = eqpool.tile([P, B * Q], BF16, tag="eq")
                eqv3 = eq[:].rearrange("p (b q) -> p b q", b=B)
                qcb = dq_sb[:, j0:j0 + B].rearrange(
                    "p (b one) -> p b one", one=1).to_broadcast([P, B, Q])
                nc.vector.tensor_tensor(out=eqv3, in0=iota3, in1=qcb,
                                        op=mybir.AluOpType.is_equal)
                eqv = eqpool.tile([P, B * Q], BF16, tag="eqv")
                eqvv3 = eqv[:].rearrange("p (b q) -> p b q", b=B)
                vcb = vD_sb[:, j0:j0 + B].rearrange(
                    "p (b one) -> p b one", one=1).to_broadcast([P, B, Q])
                nc.vector.tensor_tensor(out=eqvv3, in0=eqv3, in1=vcb,
                                        op=mybir.AluOpType.mult)
                for b in range(B):
                    j = j0 + b
                    nc.tensor.matmul(out=psS[:], lhsT=ident_sb[:],
                                     rhs=eqv[:, b * Q:(b + 1) * Q],
                                     start=(j == 0), stop=(j == CD - 1))
            t1 = pool.tile([P, Q], F32, tag="t1")
            t2 = pool.tile([P, Q], F32, tag="t2")
            nc.vector.tensor_tensor(out=t1[:], in0=xg_sb[:], in1=dinv_sb[:],
                                    op=mybir.AluOpType.mult)
            nc.vector.tensor_tensor(out=t1[:], in0=t1[:], in1=psS[:],
                                    op=mybir.AluOpType.add)
            nc.vector.tensor_tensor(out=t2[:], in0=dinv_sb[:], in1=dinv_sb[:],
                                    op=mybir.AluOpType.mult)
            nc.vector.tensor_tensor(out=t1[:], in0=t1[:], in1=t2[:],
                                    op=mybir.AluOpType.mult)
            nc.sync.dma_start(y_o.ap(), t1[:])
    nc.compile()
    return nc


def build_k5(CD, a_vec, b_vec, b2_vec):
    """sp/sm scatter from y values; alpha/beta; masked relu feature sums."""
    nc = bacc.Bacc("TRN2", target_bir_lowering=False, debug=False)
    dq = nc.dram_tensor("dq", [P, CD], BF16, kind="ExternalInput")
    vy = nc.dram_tensor("vy", [P, CD], F32, kind="ExternalInput")
    iota, ident = _consts(nc)
    dinv = nc.dram_tensor("dinvg", [P, Q], F32, kind="ExternalInput")
    yg = nc.dram_tensor("yg", [P, Q], F32, kind="ExternalInput")
    maskg = nc.dram_tensor("maskg", [P, Q], F32, kind="ExternalInput")
    acc_o = nc.dram_tensor("acc", [P, 16], F32, kind="ExternalOutput")
    with tile.TileContext(nc) as tc:
        with tc.tile_pool(name="sb", bufs=1) as pool, \
             tc.tile_pool(name="eqp", bufs=6) as eqpool, \
             tc.tile_pool(name="ps", bufs=1, space="PSUM") as psp:
            dq_sb = pool.tile([P, CD], BF16, tag="dq")
            vy_sb = pool.tile([P, CD], F32, tag="vy")
            vp_sb = pool.tile([P, CD], F32, tag="vp")
            vm_sb = pool.tile([P, CD], F32, tag="vm")
            iota_sb = pool.tile([P, Q], BF16, tag="iota")
            ident_sb = pool.tile([P, P], BF16, tag="ident")
            dinv_sb = pool.tile([P, Q], F32, tag="dinv")
            y_sb = pool.tile([P, Q], F32, tag="yg")
            mask_sb = pool.tile([P, Q], F32, tag="maskg")
            for t_sb, t in ((dq_sb, dq), (vy_sb, vy), (iota_sb, iota),
                            (ident_sb, ident), (dinv_sb, dinv),
                            (y_sb, yg), (mask_sb, maskg)):
                nc.sync.dma_start(t_sb[:], t.ap())
            # per-edge relu(y[src]), relu(-y[src]) from the exchanged y values
            nc.vector.tensor_scalar(out=vp_sb[:], in0=vy_sb[:], scalar1=0.0,
                                    scalar2=None, op0=mybir.AluOpType.max)
            nc.vector.tensor_scalar(out=vm_sb[:], in0=vy_sb[:], scalar1=-1.0,
                                    scalar2=0.0, op0=mybir.AluOpType.mult,
                                    op1=mybir.AluOpType.max)
            psP = psp.tile([P, Q], F32, space="PSUM")
            psM = psp.tile([P, Q], F32, space="PSUM")
            B = 8
            iota3 = iota_sb[:].rearrange("p (one q) -> p one q",
                                         one=1).to_broadcast([P, B, Q])
            for j0 in range(0, CD, B):
                eq = eqpool.tile([P, B * Q], BF16, tag="eq")
                eqv3 = eq[:].rearrange("p (b q) -> p b q", b=B)
                qcb = dq_sb[:, j0:j0 + B].rearrange(
                    "p (b one) -> p b one", one=1).to_broadcast([P, B, Q])
                nc.vector.tensor_tensor(out=eqv3, in0=iota3, in1=qcb,
                                        op=mybir.AluOpType.is_equal)
                eqp_ = eqpool.tile([P, B * Q], BF16, tag="eqvp")
                eqm_ = eqpool.tile([P, B * Q], BF16, tag="eqvm")
                vpb = vp_sb[:, j0:j0 + B].rearrange(
                    "p (b one) -> p b one", one=1).to_broadcast([P, B, Q])
                vmb = vm_sb[:, j0:j0 + B].rearrange(
                    "p (b one) -> p b one", one=1).to_broadcast([P, B, Q])
                nc.vector.tensor_tensor(
                    out=eqp_[:].rearrange("p (b q) -> p b q", b=B),
                    in0=eqv3, in1=vpb, op=mybir.AluOpType.mult)
                nc.vector.tensor_tensor(
                    out=eqm_[:].rearrange("p (b q) -> p b q", b=B),
                    in0=eqv3, in1=vmb, op=mybir.AluOpType.mult)
                for b in range(B):
                    j = j0 + b
                    nc.tensor.matmul(out=psP[:], lhsT=ident_sb[:],
                                     rhs=eqp_[:, b * Q:(b + 1) * Q],
                                     start=(j == 0), stop=(j == CD - 1))
                    nc.tensor.matmul(out=psM[:], lhsT=ident_sb[:],
                                     rhs=eqm_[:, b * Q:(b + 1) * Q],
                                     start=(j == 0), stop=(j == CD - 1))
            alpha = pool.tile([P, Q], F32, tag="alpha")
            beta = pool.tile([P, Q], F32, tag="beta")
            ypg = pool.tile([P, Q], F32, tag="ypg")
            ymg = pool.tile([P, Q], F32, tag="ymg")
            nc.vector.tensor_scalar(out=ypg[:], in0=y_sb[:], scalar1=0.0,
                                    scalar2=None, op0=mybir.AluOpType.max)
            nc.vector.tensor_scalar(out=ymg[:], in0=y_sb[:], scalar1=-1.0,
                                    scalar2=0.0, op0=mybir.AluOpType.mult,
                                    op1=mybir.AluOpType.max)
            nc.vector.tensor_tensor(out=alpha[:], in0=ypg[:], in1=psP[:],
                                    op=mybir.AluOpType.add)
            nc.vector.tensor_tensor(out=alpha[:], in0=alpha[:], in1=dinv_sb[:],
                                    op=mybir.AluOpType.mult)
            nc.vector.tensor_tensor(out=beta[:], in0=ymg[:], in1=psM[:],
                                    op=mybir.AluOpType.add)
            nc.vector.tensor_tensor(out=beta[:], in0=beta[:], in1=dinv_sb[:],
                                    op=mybir.AluOpType.mult)
            acc_sb = pool.tile([P, 16], F32, tag="acc")
            z = pool.tile([P, Q], F32, tag="z")
            z2 = pool.tile([P, Q], F32, tag="z2")
            for jf in range(16):
                nc.vector.tensor_scalar(out=z[:], in0=alpha[:],
                                        scalar1=float(a_vec[jf]), scalar2=None,
                                        op0=mybir.AluOpType.mult)
                nc.vector.tensor_scalar(out=z2[:], in0=beta[:],
                                        scalar1=float(b_vec[jf]),
                                        scalar2=float(b2_vec[jf]),
                                        op0=mybir.AluOpType.mult,
                                        op1=mybir.AluOpType.add)
                nc.vector.tensor_tensor(out=z[:], in0=z[:], in1=z2[:],
                                        op=mybir.AluOpType.add)
                nc.vector.tensor_scalar(out=z[:], in0=z[:], scalar1=0.0,
                                        scalar2=None, op0=mybir.AluOpType.max)
                nc.vector.tensor_tensor(out=z[:], in0=z[:], in1=mask_sb[:],
                                        op=mybir.AluOpType.mult)
                nc.vector.tensor_reduce(out=acc_sb[:, jf:jf + 1], in_=z[:],
                                        axis=mybir.AxisListType.X,
                                        op=mybir.AluOpType.add)
            nc.sync.dma_start(acc_o.ap(), acc_sb[:])
    nc.compile()
    return nc


# ---------------- pipeline ----------------

def run_pipeline(inputs, trace=False):
    x = np.asarray(inputs["x"]).reshape(-1).astype(np.float32)
    ei = np.asarray(inputs["edge_index"])
    src = ei[0].astype(np.int64)
    dst = ei[1].astype(np.int64)
    W1 = np.asarray(inputs["W1"]).astype(np.float64)[0]
    W2 = np.asarray(inputs["W2"]).astype(np.float64)
    b2 = np.asarray(inputs["b2"]).astype(np.float64)
    Wl = np.asarray(inputs["Wl"]).astype(np.float64)
    bl = np.asarray(inputs["bl"]).astype(np.float64)
    a_vec = np.maximum(W1, 0) @ W2
    b_vec = np.maximum(-W1, 0) @ W2

    xpad = np.zeros(NPAD, np.float32)
    xpad[:x.shape[0]] = x
    maskpad = np.zeros(NPAD, np.float32)
    maskpad[:x.shape[0]] = 1.0
    x_grids = _grid_of(xpad)
    mask_grids = _grid_of(maskpad)

    CD, dflat, _ = _build_layout(dst)
    CS, sflat, _ = _build_layout(src)
    k, loc = dst // NSH, dst % NSH
    dq_st = _stage_qcodes(CD, dflat, (dst % NSH) >> 7)
    sq_st = _stage_qcodes(CS, sflat, (src % NSH) >> 7)
    consts = _const_inputs()

    phase_ns = {}

    def run(nc, in_maps, name):
        res = bass_utils.run_bass_kernel_spmd(
            nc, in_maps, core_ids=list(range(NC)), trace=trace)
        phase_ns[name] = res.exec_time_ns
        return res.results

    nc1 = build_k1(CD)
    r1 = run(nc1, [dict(dq=dq_st[kk], xg=x_grids[kk], **consts)
                   for kk in range(NC)], "k1")
    dinv_g = np.stack([r1[kk]["dinv"] for kk in range(NC)])
    u_g = np.stack([r1[kk]["u"] for kk in range(NC)])

    nc2 = build_k2(CS)
    r2 = run(nc2, [dict(sq=sq_st[kk], tab0=u_g[kk], iota=consts["iota"],
                        ident=consts["ident"]) for kk in range(NC)], "k2")
    msg_flat = np.stack([r2[kk]["m0"] for kk in range(NC)]).reshape(-1)

    vD = np.zeros(NC * P * CD, np.float32)
    vD[dflat] = msg_flat[sflat]
    vD = vD.reshape(NC, P, CD)

    nc3 = build_k3(CD)
    r3 = run(nc3, [dict(dq=dq_st[kk], vD=vD[kk], dinvg=dinv_g[kk],
                        xg=x_grids[kk], **consts) for kk in range(NC)], "k3")
    y_g = np.stack([r3[kk]["yg"] for kk in range(NC)])

    nc4 = build_k2(CS)
    r4 = run(nc4, [dict(sq=sq_st[kk], tab0=y_g[kk], iota=consts["iota"],
                        ident=consts["ident"]) for kk in range(NC)], "k4")
    my_flat = np.stack([r4[kk]["m0"] for kk in range(NC)]).reshape(-1)

    vy = np.zeros(NC * P * CD, np.float32)
    vy[dflat] = my_flat[sflat]
    vy = vy.reshape(NC, P, CD)

    nc5 = build_k5(CD, a_vec, b_vec, b2)
    r5 = run(nc5, [dict(dq=dq_st[kk], vy=vy[kk], dinvg=dinv_g[kk],
                        yg=y_g[kk], maskg=mask_grids[kk], **consts)
                   for kk in range(NC)], "k5")
    acc = np.stack([r5[kk]["acc"] for kk in range(NC)])

    pooled = acc.sum(axis=(0, 1)).astype(np.float64) / float(x.shape[0])
    logits = pooled @ Wl + bl
    m = logits.max()
    out = (logits - m) - np.log(np.exp(logits - m).sum())
    return out[None, :].astype(np.float32), phase_ns


def kernel(**inputs) -> np.ndarray:
    out, _ = run_pipeline(inputs, trace=False)
    return out



# revision 6
# speedup vs baseline: 66.5241x; 66.5241x over previous
"""Trainium2 Bass kernel for nn_Classifier_8461085573484 (2-layer GCN classifier).

Math: with x [N,1] and b1=0 (structurally true for this problem), both GCN
layers collapse to scalar per-node quantities:
  deg_d = indeg(d)+1;  dinv = 1/sqrt(deg);  u = x*dinv
  S_d   = sum_{e->d} u[src];   y = dinv^2*(S + u)   (y = layer1-scalar * dinv)
  sv_d  = sum_{e->d} y[src];  sp_d = sum_{e->d} relu(y[src]);  sm = sp - sv
  alpha = dinv*(sp + relu(y));      beta = dinv*(sm + relu(-y))
  out2  = relu(alpha a^T + beta b^T + b2), a = relu(W1)@W2, b = relu(-W1)@W2
  logits = mean(out2) @ Wl + bl -> log_softmax.

Sharding (8 NeuronCores): NC k owns node range [12544k, 12544(k+1)); node
(k, lane, q) sits at lane = local%128, q = local//128 on core k.

Layout: per-edge values live in *expanded row layout* grids [P, Q*K]:
node (lane, q) owns the K-slot column segment [q*K, (q+1)*K); edge j of that
node (rank by dst or src) occupies slot j, pad slots are zero.  Segment sums
are then plain vector tensor_reduce over [P, Q, K] (no one-hot work blowup);
the neighbor "gather" is a device-side broadcast of the node grid into the
src-major expanded layout.  The host only routes / permutes per-edge values
between the src-major and dst-major layouts (no arithmetic) and applies the
O(1) classifier head.
"""
import contextlib
import ctypes
import sys
import types

import numpy as np

from concourse import bacc, bass, mybir
import concourse.tile as tile
from concourse import bass_utils

P = 128
Q = 98
NSH = P * Q            # 12544 nodes per NC shard
NC = 8
NPAD = NSH * NC        # 100352
N = 100000
F32 = mybir.dt.float32
BF16 = mybir.dt.bfloat16
QC = 14                # q-chunk size (Q = 7 chunks of 14)
EXCH_BF16 = True       # exchange per-edge values in bf16 (halves DMA traffic)


def _install_ntff_shim():
    """Provide antenv.axon_hooks so run_bass_kernel_spmd(trace=True) works."""
    if "antenv.axon_hooks" in sys.modules:
        return
    import antenv

    _hook = None
    try:
        lib = ctypes.CDLL("/opt/axon/libaxon_pjrt.so")
        if hasattr(lib, "axon_start_nrt_profile"):
            lib.axon_start_nrt_profile.argtypes = [
                ctypes.POINTER(ctypes.c_int64), ctypes.c_size_t]
            lib.axon_start_nrt_profile.restype = ctypes.c_int64
            lib.axon_stop_nrt_profile.argtypes = [ctypes.c_char_p]
            lib.axon_stop_nrt_profile.restype = ctypes.c_int64

            @contextlib.contextmanager
            def _hook_impl(output_dir, device_ids):
                import jax
                jax.devices()
                if device_ids:
                    ids = (ctypes.c_int64 * len(device_ids))(*device_ids)
                    rc = lib.axon_start_nrt_profile(ids, len(device_ids))
                else:
                    rc = lib.axon_start_nrt_profile(None, 0)
                if rc != 0:
                    raise RuntimeError(f"axon_start_nrt_profile rc={rc}")
                try:
                    yield
                finally:
                    n = lib.axon_stop_nrt_profile(str(output_dir).encode())
                    if n < 0:
                        raise RuntimeError(f"axon_stop_nrt_profile rc={n}")

            _hook = _hook_impl
    except OSError:
        pass

    mod = types.ModuleType("antenv.axon_hooks")
    mod._hook = _hook
    mod.get_axon_ntff_profile_hook = lambda: mod._hook

    def set_axon_ntff_profile_hook(h):
        mod._hook = h

    mod.set_axon_ntff_profile_hook = set_axon_ntff_profile_hook
    sys.modules["antenv.axon_hooks"] = mod
    antenv.axon_hooks = mod


_install_ntff_shim()


# ---------------- host routing (sharding/layout only, no arithmetic) -------

def _ranks(keys):
    """Rank of each edge within its node group, plus per-node counts."""
    counts = np.bincount(keys, minlength=NPAD).astype(np.int64)
    starts = np.zeros(NPAD, np.int64)
    starts[1:] = np.cumsum(counts)[:-1]
    order = np.argsort(keys, kind="stable")
    rank = np.empty(keys.shape[0], np.int64)
    rank[order] = np.arange(keys.shape[0], dtype=np.int64) - starts[keys[order]]
    return rank, int(counts.max())


def _slots(nodes, rank, K):
    """Flat index into [NC, P, Q*K] expanded layout for (node, rank)."""
    k = nodes // NSH
    loc = nodes - k * NSH
    lane = loc % P
    q = loc >> 7
    return ((k * P + lane) * Q + q) * K + rank


def _grid_of(vec_padded):
    return np.ascontiguousarray(vec_padded.reshape(NC, Q, P).transpose(0, 2, 1))


# ---------------- device phase builders ----------------

def _exch_dt():
    return BF16 if EXCH_BF16 else F32


def build_pA(KD, KS):
    """indeg via mask row-reduce -> dinv, u; broadcast u to src-major m1."""
    EXT = _exch_dt()
    nc = bacc.Bacc("TRN2", target_bir_lowering=False, debug=False)
    maskD = nc.dram_tensor("maskD", [P, Q * KD], BF16, kind="ExternalInput")
    xg = nc.dram_tensor("xg", [P, Q], F32, kind="ExternalInput")
    dinv_o = nc.dram_tensor("dinv", [P, Q], F32, kind="ExternalOutput")
    u_o = nc.dram_tensor("u", [P, Q], F32, kind="ExternalOutput")
    m1_o = nc.dram_tensor("m1", [P, Q * KS], EXT, kind="ExternalOutput")
    with tile.TileContext(nc) as tc:
        with tc.tile_pool(name="sb", bufs=1) as pool, \
             tc.tile_pool(name="inp", bufs=3) as inp, \
             tc.tile_pool(name="outp", bufs=3) as outp:
            xg_sb = pool.tile([P, Q], F32, tag="xg")
            nc.sync.dma_start(xg_sb[:], xg.ap())
            indeg = pool.tile([P, Q], F32, tag="indeg")
            dinv_sb = pool.tile([P, Q], F32, tag="dinv")
            u_sb = pool.tile([P, Q], F32, tag="u")
            sq_sb = pool.tile([P, Q], F32, tag="sq")
            ones_sb = pool.tile([P, QC * KS], EXT, tag="ones")
            nc.vector.memset(ones_sb[:], 1.0)
            ones3 = ones_sb[:].rearrange("p (q k) -> p q k", k=KS)
            for c0 in range(0, Q, QC):
                mt = inp.tile([P, QC * KD], BF16, tag="mchunk")
                nc.sync.dma_start(mt[:], maskD.ap()[:, c0 * KD:(c0 + QC) * KD])
                nc.vector.tensor_reduce(
                    out=indeg[:, c0:c0 + QC],
                    in_=mt[:].rearrange("p (q k) -> p q k", k=KD),
                    axis=mybir.AxisListType.X, op=mybir.AluOpType.add)
                nc.scalar.activation(
                    out=sq_sb[:, c0:c0 + QC], in_=indeg[:, c0:c0 + QC],
                    func=mybir.ActivationFunctionType.Sqrt, bias=1.0, scale=1.0)
                nc.vector.reciprocal(out=dinv_sb[:, c0:c0 + QC],
                                     in_=sq_sb[:, c0:c0 + QC])
                nc.vector.tensor_tensor(
                    out=u_sb[:, c0:c0 + QC], in0=xg_sb[:, c0:c0 + QC],
                    in1=dinv_sb[:, c0:c0 + QC], op=mybir.AluOpType.mult)
                bt = outp.tile([P, QC * KS], EXT, tag="bchunk")
                nc.vector.tensor_tensor(
                    out=bt[:].rearrange("p (q k) -> p q k", k=KS),
                    in0=ones3,
                    in1=u_sb[:, c0:c0 + QC].rearrange(
                        "p (q one) -> p q one", one=1).to_broadcast([P, QC, KS]),
                    op=mybir.AluOpType.mult)
                nc.sync.dma_start(m1_o.ap()[:, c0 * KS:(c0 + QC) * KS], bt[:])
            nc.sync.dma_start(dinv_o.ap(), dinv_sb[:])
            nc.sync.dma_start(u_o.ap(), u_sb[:])
    nc.compile()
    return nc


def build_pB(KD, KS):
    """S = segsum(vD1); y = dinv^2 * (S + u); broadcast y to src-major m2."""
    EXT = _exch_dt()
    nc = bacc.Bacc("TRN2", target_bir_lowering=False, debug=False)
    vD1 = nc.dram_tensor("vD1", [P, Q * KD], EXT, kind="ExternalInput")
    u_i = nc.dram_tensor("u", [P, Q], F32, kind="ExternalInput")
    dinv_i = nc.dram_tensor("dinvg", [P, Q], F32, kind="ExternalInput")
    y_o = nc.dram_tensor("yg", [P, Q], F32, kind="ExternalOutput")
    m2_o = nc.dram_tensor("m2", [P, Q * KS], EXT, kind="ExternalOutput")
    with tile.TileContext(nc) as tc:
        with tc.tile_pool(name="sb", bufs=1) as pool, \
             tc.tile_pool(name="inp", bufs=3) as inp, \
             tc.tile_pool(name="outp", bufs=3) as outp:
            u_sb = pool.tile([P, Q], F32, tag="u")
            dinv_sb = pool.tile([P, Q], F32, tag="dinv")
            d2_sb = pool.tile([P, Q], F32, tag="d2")
            s_sb = pool.tile([P, Q], F32, tag="s")
            y_sb = pool.tile([P, Q], F32, tag="y")
            nc.sync.dma_start(u_sb[:], u_i.ap())
            nc.sync.dma_start(dinv_sb[:], dinv_i.ap())
            nc.vector.tensor_tensor(out=d2_sb[:], in0=dinv_sb[:],
                                    in1=dinv_sb[:], op=mybir.AluOpType.mult)
            ones_sb = pool.tile([P, QC * KS], EXT, tag="ones")
            nc.vector.memset(ones_sb[:], 1.0)
            ones3 = ones_sb[:].rearrange("p (q k) -> p q k", k=KS)
            for c0 in range(0, Q, QC):
                vt = inp.tile([P, QC * KD], EXT, tag="vchunk")
                nc.sync.dma_start(vt[:], vD1.ap()[:, c0 * KD:(c0 + QC) * KD])
                nc.vector.tensor_reduce(
                    out=s_sb[:, c0:c0 + QC],
                    in_=vt[:].rearrange("p (q k) -> p q k", k=KD),
                    axis=mybir.AxisListType.X, op=mybir.AluOpType.add)
                nc.vector.tensor_tensor(
                    out=s_sb[:, c0:c0 + QC], in0=s_sb[:, c0:c0 + QC],
                    in1=u_sb[:, c0:c0 + QC], op=mybir.AluOpType.add)
                nc.vector.tensor_tensor(
                    out=y_sb[:, c0:c0 + QC], in0=s_sb[:, c0:c0 + QC],
                    in1=d2_sb[:, c0:c0 + QC], op=mybir.AluOpType.mult)
                bt = outp.tile([P, QC * KS], EXT, tag="bchunk")
                nc.vector.tensor_tensor(
                    out=bt[:].rearrange("p (q k) -> p q k", k=KS),
                    in0=ones3,
                    in1=y_sb[:, c0:c0 + QC].rearrange(
                        "p (q one) -> p q one", one=1).to_broadcast([P, QC, KS]),
                    op=mybir.AluOpType.mult)
                nc.sync.dma_start(m2_o.ap()[:, c0 * KS:(c0 + QC) * KS], bt[:])
            nc.sync.dma_start(y_o.ap(), y_sb[:])
    nc.compile()
    return nc


def build_pC(KD):
    """sp/sm segsums of relu'd y messages; alpha/beta; 16-feature sums."""
    EXT = _exch_dt()
    nc = bacc.Bacc("TRN2", target_bir_lowering=False, debug=False)
    vD2 = nc.dram_tensor("vD2", [P, Q * KD], EXT, kind="ExternalInput")
    dinv_i = nc.dram_tensor("dinvg", [P, Q], F32, kind="ExternalInput")
    y_i = nc.dram_tensor("yg", [P, Q], F32, kind="ExternalInput")
    mask_i = nc.dram_tensor("maskg", [P, Q], F32, kind="ExternalInput")
    cvec = nc.dram_tensor("cvec", [P, 48], F32, kind="ExternalInput")
    acc_o = nc.dram_tensor("acc", [P, 16], F32, kind="ExternalOutput")
    with tile.TileContext(nc) as tc:
        with tc.tile_pool(name="sb", bufs=1) as pool, \
             tc.tile_pool(name="inp", bufs=3) as inp, \
             tc.tile_pool(name="rel", bufs=3) as relp:
            dinv_sb = pool.tile([P, Q], F32, tag="dinv")
            y_sb = pool.tile([P, Q], F32, tag="y")
            mask_sb = pool.tile([P, Q], F32, tag="mask")
            cvec_sb = pool.tile([P, 48], F32, tag="cvec")
            sv_sb = pool.tile([P, Q], F32, tag="sv")
            sp_sb = pool.tile([P, Q], F32, tag="sp")
            for t_sb, t in ((dinv_sb, dinv_i), (y_sb, y_i),
                            (mask_sb, mask_i), (cvec_sb, cvec)):
                nc.sync.dma_start(t_sb[:], t.ap())
            for c0 in range(0, Q, QC):
                vt = inp.tile([P, QC * KD], EXT, tag="vchunk")
                nc.sync.dma_start(vt[:], vD2.ap()[:, c0 * KD:(c0 + QC) * KD])
                nc.vector.tensor_reduce(
                    out=sv_sb[:, c0:c0 + QC],
                    in_=vt[:].rearrange("p (q k) -> p q k", k=KD),
                    axis=mybir.AxisListType.X, op=mybir.AluOpType.add)
                rt = relp.tile([P, QC * KD], EXT, tag="rchunk")
                nc.scalar.activation(out=rt[:], in_=vt[:],
                                     func=mybir.ActivationFunctionType.Relu)
                nc.vector.tensor_reduce(
                    out=sp_sb[:, c0:c0 + QC],
                    in_=rt[:].rearrange("p (q k) -> p q k", k=KD),
                    axis=mybir.AxisListType.X, op=mybir.AluOpType.add)
            # node-side terms
            yp = pool.tile([P, Q], F32, tag="yp")
            ym = pool.tile([P, Q], F32, tag="ym")
            alpha = pool.tile([P, Q], F32, tag="alpha")
            beta = pool.tile([P, Q], F32, tag="beta")
            nc.scalar.activation(out=yp[:], in_=y_sb[:],
                                 func=mybir.ActivationFunctionType.Relu)
            nc.vector.tensor_tensor(out=ym[:], in0=yp[:], in1=y_sb[:],
                                    op=mybir.AluOpType.subtract)
            nc.vector.tensor_tensor(out=alpha[:], in0=sp_sb[:], in1=yp[:],
                                    op=mybir.AluOpType.add)
            nc.vector.tensor_tensor(out=alpha[:], in0=alpha[:], in1=dinv_sb[:],
                                    op=mybir.AluOpType.mult)
            nc.vector.tensor_tensor(out=sv_sb[:], in0=sp_sb[:], in1=sv_sb[:],
                                    op=mybir.AluOpType.subtract)  # sm
            nc.vector.tensor_tensor(out=beta[:], in0=sv_sb[:], in1=ym[:],
                                    op=mybir.AluOpType.add)
            nc.vector.tensor_tensor(out=beta[:], in0=beta[:], in1=dinv_sb[:],
                                    op=mybir.AluOpType.mult)
            # 16 features at once in [P, 16, Q] layout
            t1 = pool.tile([P, 16 * Q], F32, tag="t1")
            t2 = pool.tile([P, 16 * Q], F32, tag="t2")
            t13 = t1[:].rearrange("p (j q) -> p j q", j=16)
            t23 = t2[:].rearrange("p (j q) -> p j q", j=16)
            alpha_b = alpha[:].rearrange("p (one q) -> p one q",
                                         one=1).to_broadcast([P, 16, Q])
            beta_b = beta[:].rearrange("p (one q) -> p one q",
                                       one=1).to_broadcast([P, 16, Q])
            mask_b = mask_sb[:].rearrange("p (one q) -> p one q",
                                          one=1).to_broadcast([P, 16, Q])
            a_b = cvec_sb[:, 0:16].rearrange("p (j one) -> p j one",
                                             one=1).to_broadcast([P, 16, Q])
            b_b = cvec_sb[:, 16:32].rearrange("p (j one) -> p j one",
                                              one=1).to_broadcast([P, 16, Q])
            b2_b = cvec_sb[:, 32:48].rearrange("p (j one) -> p j one",
                                               one=1).to_broadcast([P, 16, Q])
            nc.vector.tensor_tensor(out=t13, in0=alpha_b, in1=a_b,
                                    op=mybir.AluOpType.mult)
            nc.vector.tensor_tensor(out=t23, in0=beta_b, in1=b_b,
                                    op=mybir.AluOpType.mult)
            nc.vector.tensor_tensor(out=t13, in0=t13, in1=t23,
                                    op=mybir.AluOpType.add)
            nc.vector.tensor_tensor(out=t13, in0=t13, in1=b2_b,
                                    op=mybir.AluOpType.add)
            nc.scalar.activation(out=t1[:], in_=t1[:],
                                 func=mybir.ActivationFunctionType.Relu)
            nc.vector.tensor_tensor(out=t13, in0=t13, in1=mask_b,
                                    op=mybir.AluOpType.mult)
            acc_sb = pool.tile([P, 16], F32, tag="acc")
            nc.vector.tensor_reduce(out=acc_sb[:], in_=t13,
                                    axis=mybir.AxisListType.X,
                                    op=mybir.AluOpType.add)
            nc.sync.dma_start(acc_o.ap(), acc_sb[:])
    nc.compile()
    return nc


# ---------------- pipeline ----------------

def run_pipeline(inputs, trace=False):
    import ml_dtypes
    exch_np = ml_dtypes.bfloat16 if EXCH_BF16 else np.float32

    x = np.asarray(inputs["x"]).reshape(-1).astype(np.float32)
    ei = np.asarray(inputs["edge_index"])
    src = ei[0].astype(np.int64)
    dst = ei[1].astype(np.int64)
    W1 = np.asarray(inputs["W1"]).astype(np.float64)[0]
    W2 = np.asarray(inputs["W2"]).astype(np.float64)
    b2 = np.asarray(inputs["b2"]).astype(np.float64)
    Wl = np.asarray(inputs["Wl"]).astype(np.float64)
    bl = np.asarray(inputs["bl"]).astype(np.float64)
    a_vec = np.maximum(W1, 0) @ W2
    b_vec = np.maximum(-W1, 0) @ W2

    xpad = np.zeros(NPAD, np.float32)
    xpad[:x.shape[0]] = x
    maskpad = np.zeros(NPAD, np.float32)
    maskpad[:x.shape[0]] = 1.0
    x_grids = _grid_of(xpad)
    mask_grids = _grid_of(maskpad)

    rank_d, KD = _ranks(dst)
    rank_s, KS = _ranks(src)
    KD += KD & 1   # even widths
    KS += KS & 1
    dslot = _slots(dst, rank_d, KD)
    sslot = _slots(src, rank_s, KS)

    maskD = np.zeros(NC * P * Q * KD, np.float32)
    maskD[dslot] = 1.0
    maskD = np.ascontiguousarray(
        maskD.reshape(NC, P, Q * KD).astype(ml_dtypes.bfloat16))

    cvec = np.zeros(48, np.float32)
    cvec[0:16] = a_vec
    cvec[16:32] = b_vec
    cvec[32:48] = b2
    cvec = np.ascontiguousarray(np.tile(cvec, (P, 1)))

    phase_ns = {}

    def run(nc, in_maps, name):
        res = bass_utils.run_bass_kernel_spmd(
            nc, in_maps, core_ids=list(range(NC)), trace=trace)
        phase_ns[name] = res.exec_time_ns
        return res.results

    def exchange(m_stack):
        """Permute per-edge values: src-major layout -> dst-major layout."""
        m_flat = np.ascontiguousarray(m_stack).reshape(-1)
        v = np.zeros(NC * P * Q * KD, exch_np)
        v[dslot] = m_flat[sslot]
        return v.reshape(NC, P, Q * KD)

    ncA = build_pA(KD, KS)
    rA = run(ncA, [dict(maskD=maskD[kk], xg=x_grids[kk]) for kk in range(NC)],
             "pA")
    dinv_g = np.stack([rA[kk]["dinv"] for kk in range(NC)])
    u_g = np.stack([rA[kk]["u"] for kk in range(NC)])
    vD1 = exchange(np.stack([rA[kk]["m1"] for kk in range(NC)]))

    ncB = build_pB(KD, KS)
    rB = run(ncB, [dict(vD1=vD1[kk], u=u_g[kk], dinvg=dinv_g[kk])
                   for kk in range(NC)], "pB")
    y_g = np.stack([rB[kk]["yg"] for kk in range(NC)])
    vD2 = exchange(np.stack([rB[kk]["m2"] for kk in range(NC)]))

    ncC = build_pC(KD)
    rC = run(ncC, [dict(vD2=vD2[kk], dinvg=dinv_g[kk], yg=y_g[kk],
                        maskg=mask_grids[kk], cvec=cvec)
                   for kk in range(NC)], "pC")
    acc = np.stack([rC[kk]["acc"] for kk in range(NC)])

    pooled = acc.sum(axis=(0, 1)).astype(np.float64) / float(x.shape[0])
    logits = pooled @ Wl + bl
    m = logits.max()
    out = (logits - m) - np.log(np.exp(logits - m).sum())
    return out[None, :].astype(np.float32), phase_ns


def kernel(**inputs) -> np.ndarray:
    out, _ = run_pipeline(inputs, trace=False)
    return out
